# revision 4
# baseline (speedup 1.0000x reference)
"""Multi-head attention (B=2, L=2048, D=1024, H=16) on 8 Trainium2 NeuronCores.

Sharding: tensor-parallel over heads. Core c owns heads 2c, 2c+1, i.e. rows
[128c, 128c+128) of Wq/Wk/Wv and columns [128c, 128c+128) of Wo. Each core
computes Q/K/V projections for its 128 channels over all 4096 tokens,
attention for its 2 heads (both batches), and a partial out-projection
y_c = attnO_c @ Wo[:, sl].T. The host sums the 8 partials and adds bo
(the gather/unshard step).

Device-side layout notes:
- Activations arrive pre-transposed (host): qT/kT/vT are [D, B*L] so the
  contraction dim d lands on SBUF partitions without any on-device transpose.
- Scores are computed transposed (S.T tiles [k,q]) so softmax P.T lands in
  SBUF ready to be the PV matmul's moving operand; softmax-over-partitions is
  avoided by skipping the max-subtraction (scores are ~N(0,1); exp cannot
  overflow fp32) and computing row sums r with ones-matmuls on the PE.
- The key-padding mask folds into the exp: bias is 0 / -30000 per k-token,
  added per-partition by the ACT instruction, so masked keys exp to 0.0.
- PV packs the two heads into one PSUM bank via column tiling; since
  start=True clears has_written for the whole bank, shared banks are
  pre-cleared with a zero dummy matmul and all real matmuls accumulate.
- Normalization 1/r is broadcast across partitions with a small K=33 matmul
  (compute engines cannot move data across partitions).
"""

import os
import sys

for _p in ("/opt/trn_rl_repo", os.path.expanduser("~/.axon_site/_ro/trn_rl_repo")):
    if os.path.isdir(_p) and _p not in sys.path:
        sys.path.insert(0, _p)

import numpy as np

B = 2
L = 2048
D = 1024
T = B * L            # 4096 tokens
E = 128              # channels per core (2 heads x 64)
HD = 64              # head dim
N_CORES = 8
SCALE = 1.0 / 8.0    # 1/sqrt(HD)
MASK_BIAS = -30000.0

N_TT = T // 512      # 8 projection token tiles
N_DC = D // 128      # 8 contraction chunks
N_QT = L // 512      # 4 query tiles per batch
N_KT = L // 128      # 16 key tiles per batch

_cached = {}


def _build_program(has_bq, has_bk, has_bv):
    import concourse.bacc as bacc
    import concourse.mybir as mybir
    import concourse.tile as tile

    F32 = mybir.dt.float32
    AF = mybir.ActivationFunctionType
    ALU = mybir.AluOpType

    nc = bacc.Bacc("TRN2", target_bir_lowering=False, debug=False, num_devices=N_CORES)

    qT = nc.dram_tensor("qT", [D, T], F32, kind="ExternalInput").ap()
    kT = nc.dram_tensor("kT", [D, T], F32, kind="ExternalInput").ap()
    vT = nc.dram_tensor("vT", [D, T], F32, kind="ExternalInput").ap()
    wq = nc.dram_tensor("wq", [D, E], F32, kind="ExternalInput").ap()
    wk = nc.dram_tensor("wk", [D, E], F32, kind="ExternalInput").ap()
    wv = nc.dram_tensor("wv", [D, E], F32, kind="ExternalInput").ap()
    wo = nc.dram_tensor("wo", [E, D], F32, kind="ExternalInput").ap()
    mbd = nc.dram_tensor("mb", [128, B * N_KT], F32, kind="ExternalInput").ap()
    onesd = nc.dram_tensor("ones", [128, 1], F32, kind="ExternalInput").ap()
    seld = nc.dram_tensor("sel2", [33, 128], F32, kind="ExternalInput").ap()
    identd = nc.dram_tensor("ident", [128, 128], F32, kind="ExternalInput").ap()
    zrowd = nc.dram_tensor("zrow", [1, 512], F32, kind="ExternalInput").ap()
    bias_d = {}
    if has_bq:
        bias_d["q"] = nc.dram_tensor("bq", [128, 1], F32, kind="ExternalInput").ap()
    if has_bk:
        bias_d["k"] = nc.dram_tensor("bk", [128, 1], F32, kind="ExternalInput").ap()
    if has_bv:
        bias_d["v"] = nc.dram_tensor("bv", [128, 1], F32, kind="ExternalInput").ap()
    yd = nc.dram_tensor("y", [T, D], F32, kind="ExternalOutput").ap()

    with tile.TileContext(nc) as tc:
        import contextlib
        with contextlib.ExitStack() as ctx:
            const = ctx.enter_context(tc.tile_pool(name="const", bufs=1))
            big = ctx.enter_context(tc.tile_pool(name="big", bufs=1))
            stg = ctx.enter_context(tc.tile_pool(name="stg", bufs=3))
            work = ctx.enter_context(tc.tile_pool(name="work", bufs=3))
            pt_pool = ctx.enter_context(tc.tile_pool(name="ptp", bufs=3))
            psum = ctx.enter_context(tc.tile_pool(name="psum", bufs=4, space="PSUM"))
            psacc = ctx.enter_context(tc.tile_pool(name="psacc", bufs=2, space="PSUM"))
            psr = ctx.enter_context(tc.tile_pool(name="psr", bufs=2, space="PSUM"))

            # ---- constants / weights ----
            w_sb = {}
            for nm, src in (("q", wq), ("k", wk), ("v", wv)):
                w = const.tile([128, D], F32, name=f"w{nm}_sb")
                for dc in range(N_DC):
                    nc.sync.dma_start(w[:, dc * 128:(dc + 1) * 128],
                                      src[dc * 128:(dc + 1) * 128, :])
                w_sb[nm] = w
            wo_sb = const.tile([128, D], F32, name="wo_sb")
            nc.sync.dma_start(wo_sb[:], wo[:])
            mb_sb = const.tile([128, B * N_KT], F32, name="mb_sb")
            nc.sync.dma_start(mb_sb[:], mbd[:])
            ones_sb = const.tile([128, 1], F32, name="ones_sb")
            nc.sync.dma_start(ones_sb[:], onesd[:])
            sel_sb = const.tile([33, 128], F32, name="sel_sb")
            nc.sync.dma_start(sel_sb[:], seld[:])
            ident_sb = const.tile([128, 128], F32, name="ident_sb")
            nc.sync.dma_start(ident_sb[:], identd[:])
            zrow_sb = const.tile([1, 512], F32, name="zrow_sb")
            nc.sync.dma_start(zrow_sb[:], zrowd[:])
            b_sb = {}
            for nm, src in bias_d.items():
                bt = const.tile([128, 1], F32, name=f"b{nm}_sb")
                nc.sync.dma_start(bt[:], src[:])
                b_sb[nm] = bt

            # ---- persistent activations (feature-major) ----
            QT = big.tile([128, T], F32, name="QT")   # [e, tok]
            KT = big.tile([128, T], F32, name="KT")   # [e, tok]
            V = big.tile([128, T], F32, name="V")     # [tok%128, 128*g + e]
            OT = big.tile([128, T], F32, name="OT")   # [e, tok] normalized attn out
            rr = big.tile([128, 512], F32, name="rr")  # softmax denoms at rows 0/32
            nc.gpsimd.memset(rr[:], 0.0)

            # ---- projections ----
            srcs = {"q": qT, "k": kT, "v": vT}
            for nm in ("q", "k", "v"):
                for tt in range(N_TT):
                    stgs = []
                    for dc in range(N_DC):
                        s = stg.tile([128, 512], F32, name=f"stg_{nm}_{tt}_{dc}",
                                     tag=f"stg{dc}")
                        nc.sync.dma_start(
                            s[:], srcs[nm][dc * 128:(dc + 1) * 128,
                                           tt * 512:(tt + 1) * 512])
                        stgs.append(s)
                    ps = psum.tile([128, 512], F32, name=f"ps_{nm}_{tt}", tag="mm")
                    for dc in range(N_DC):
                        nc.tensor.matmul(ps[:],
                                         w_sb[nm][:, dc * 128:(dc + 1) * 128],
                                         stgs[dc][:],
                                         start=(dc == 0), stop=(dc == N_DC - 1))
                    if nm in ("q", "k"):
                        dst = (QT if nm == "q" else KT)[:, tt * 512:(tt + 1) * 512]
                        if nm in b_sb:
                            nc.vector.tensor_scalar(dst, ps[:], b_sb[nm][:, 0:1],
                                                    None, ALU.add)
                        else:
                            nc.vector.tensor_copy(dst, ps[:])
                    else:
                        vs = work.tile([128, 512], F32, name=f"vs_{tt}", tag="vs")
                        nc.vector.tensor_copy(vs[:], ps[:])
                        for si in range(4):
                            vtp = psum.tile([128, 128], F32,
                                            name=f"vtp_{tt}_{si}", tag="mm")
                            nc.tensor.transpose(vtp[:],
                                                vs[:, si * 128:(si + 1) * 128],
                                                ident_sb[:])
                            g = tt * 4 + si
                            nc.vector.tensor_copy(V[:, g * 128:(g + 1) * 128],
                                                  vtp[:])

            # ---- attention (per batch, per 512-query tile) ----
            for b in range(B):
                for qt in range(N_QT):
                    q0 = b * L + qt * 512
                    ot_ps = psacc.tile([128, 512], F32, name=f"ot_{b}_{qt}",
                                       tag="acc")
                    r_ps = psr.tile([33, 512], F32, name=f"r_{b}_{qt}", tag="r")
                    # pre-clear shared accumulator banks (sets has_written
                    # everywhere so all real matmuls can accumulate)
                    nc.tensor.matmul(ot_ps[:], sel_sb[0:1, 0:128], zrow_sb[:],
                                     start=True, stop=False,
                                     skip_group_check=True)
                    nc.tensor.matmul(r_ps[:], sel_sb[0:1, 0:33], zrow_sb[:],
                                     start=True, stop=False,
                                     skip_group_check=True)
                    for kt in range(N_KT):
                        k0 = b * L + kt * 128
                        g = b * N_KT + kt
                        st = []
                        for h in range(2):
                            s = psum.tile([128, 512], F32,
                                          name=f"st_{b}_{qt}_{kt}_{h}", tag="mm")
                            nc.tensor.matmul(
                                s[:],
                                KT[h * 64:(h + 1) * 64, k0:k0 + 128],
                                QT[h * 64:(h + 1) * 64, q0:q0 + 512],
                                start=True, stop=True)
                            st.append(s)
                        pt = []
                        for h in range(2):
                            p = pt_pool.tile([128, 512], F32,
                                             name=f"pt_{b}_{qt}_{kt}_{h}",
                                             tag=f"pt{h}")
                            nc.scalar.activation(p[:], st[h][:], AF.Exp,
                                                 bias=mb_sb[:, g:g + 1],
                                                 scale=SCALE)
                            pt.append(p)
                        last = (kt == N_KT - 1)
                        for h in range(2):
                            nc.tensor.matmul(
                                ot_ps[h * 64:(h + 1) * 64, :],
                                V[:, g * 128 + h * 64: g * 128 + (h + 1) * 64],
                                pt[h][:], start=False, stop=last,
                                skip_group_check=True)
                            nc.tensor.matmul(
                                r_ps[h * 32:h * 32 + 1, :],
                                ones_sb[:, 0:1],
                                pt[h][:], start=False, stop=last,
                                skip_group_check=True)
                    # normalize: OT = ot_ps * broadcast(1/r) (+ bv)
                    nc.vector.reciprocal(rr[0:1, :], r_ps[0:1, :])
                    nc.vector.reciprocal(rr[32:33, :], r_ps[32:33, :])
                    bc_ps = psum.tile([128, 512], F32, name=f"bc_{b}_{qt}",
                                      tag="mm")
                    nc.tensor.matmul(bc_ps[:], sel_sb[:], rr[0:33, :],
                                     start=True, stop=True)
                    bc_sb = work.tile([128, 512], F32, name=f"bcs_{b}_{qt}",
                                      tag="bcs")
                    nc.vector.tensor_copy(bc_sb[:], bc_ps[:])
                    dst = OT[:, q0:q0 + 512]
                    nc.vector.tensor_mul(dst, ot_ps[:], bc_sb[:])
                    if "v" in b_sb:
                        nc.vector.tensor_scalar(dst, dst, b_sb["v"][:, 0:1],
                                                None, ALU.add)

            # ---- partial out-projection: y = O @ Wo_c.T  (token-major) ----
            for g in range(T // 128):
                for nn in range(2):
                    yp = psum.tile([128, 512], F32, name=f"yp_{g}_{nn}", tag="mm")
                    nc.tensor.matmul(yp[:], OT[:, g * 128:(g + 1) * 128],
                                     wo_sb[:, nn * 512:(nn + 1) * 512],
                                     start=True, stop=True)
                    ys = work.tile([128, 512], F32, name=f"ys_{g}_{nn}", tag="ys")
                    nc.vector.tensor_copy(ys[:], yp[:])
                    nc.sync.dma_start(
                        yd[g * 128:(g + 1) * 128, nn * 512:(nn + 1) * 512], ys[:])

    nc.compile()
    return nc


def _host_prep(q, k, v, mask, Wq, bq, Wk, bk, Wv, bv, Wo):
    """Build the per-core input maps."""
    f32 = np.float32
    qT = np.ascontiguousarray(q.reshape(T, D).T.astype(f32))
    kT = np.ascontiguousarray(k.reshape(T, D).T.astype(f32))
    vT = np.ascontiguousarray(v.reshape(T, D).T.astype(f32))
    mb = np.where(mask, f32(MASK_BIAS), f32(0.0)).astype(f32)      # [B, L]
    mb = np.ascontiguousarray(
        np.transpose(mb.reshape(B, N_KT, 128), (2, 0, 1)).reshape(128, B * N_KT))
    ones = np.ones((128, 1), f32)
    sel2 = np.zeros((33, 128), f32)
    sel2[0, 0:64] = 1.0
    sel2[32, 64:128] = 1.0
    ident = np.eye(128, dtype=f32)
    zrow = np.zeros((1, 512), f32)

    in_maps = []
    for c in range(N_CORES):
        sl = slice(c * E, (c + 1) * E)
        m = {
            "qT": qT, "kT": kT, "vT": vT,
            "wq": np.ascontiguousarray(Wq[sl, :].T.astype(f32)),
            "wk": np.ascontiguousarray(Wk[sl, :].T.astype(f32)),
            "wv": np.ascontiguousarray(Wv[sl, :].T.astype(f32)),
            "wo": np.ascontiguousarray(Wo[:, sl].T.astype(f32)),
            "mb": mb, "ones": ones, "sel2": sel2, "ident": ident, "zrow": zrow,
        }
        if np.any(bq):
            m["bq"] = np.ascontiguousarray(bq[sl].astype(f32).reshape(128, 1))
        if np.any(bk):
            m["bk"] = np.ascontiguousarray(bk[sl].astype(f32).reshape(128, 1))
        if np.any(bv):
            m["bv"] = np.ascontiguousarray(bv[sl].astype(f32).reshape(128, 1))
        in_maps.append(m)
    return in_maps


def _build_floor_program():
    """Near-empty program used to measure the axon dispatch floor."""
    import concourse.bacc as bacc
    import concourse.mybir as mybir
    import concourse.tile as tile
    import contextlib

    F32 = mybir.dt.float32
    nc = bacc.Bacc("TRN2", target_bir_lowering=False, debug=False,
                   num_devices=N_CORES)
    x = nc.dram_tensor("x", [128, 8], F32, kind="ExternalInput").ap()
    y = nc.dram_tensor("yf", [128, 8], F32, kind="ExternalOutput").ap()
    with tile.TileContext(nc) as tc:
        with contextlib.ExitStack() as ctx:
            sb = ctx.enter_context(tc.tile_pool(name="sb", bufs=1))
            t = sb.tile([128, 8], F32, name="t")
            nc.sync.dma_start(t[:], x[:])
            nc.sync.dma_start(y[:], t[:])
    nc.compile()
    return nc


def _make_timed_runner(nc, in_maps):
    """Build a reusable jitted runner for `nc` (no output donation — the
    program writes every output element, so uninit result buffers are fine).
    Returns (run_once() -> per-core outputs as numpy, time_iters(n) -> [sec])."""
    import jax
    import time
    import concourse.mybir as mybir
    from concourse import bass2jax
    from jax.experimental.shard_map import shard_map
    from jax.sharding import Mesh, NamedSharding, PartitionSpec

    bass2jax.install_neuronx_cc_hook()

    partition_name = nc.partition_id_tensor.name if nc.partition_id_tensor else None
    in_names, out_names, out_avals, zero_outs = [], [], [], []
    for alloc in nc.m.functions[0].allocations:
        if not isinstance(alloc, mybir.MemoryLocationSet):
            continue
        name = alloc.memorylocations[0].name
        if alloc.kind == "ExternalInput":
            if name != partition_name:
                in_names.append(name)
        elif alloc.kind == "ExternalOutput":
            shape = tuple(alloc.tensor_shape)
            dtype = mybir.dt.np(alloc.dtype)
            out_names.append(name)
            out_avals.append(jax.core.ShapedArray(shape, dtype))
            zero_outs.append(np.zeros(shape, dtype))
    n_params = len(in_names)
    all_in_names = list(in_names) + list(out_names)
    if partition_name is not None:
        all_in_names.append(partition_name)

    def _body(*args):
        operands = list(args)
        if partition_name is not None:
            operands.append(bass2jax.partition_id_tensor())
        outs = bass2jax._bass_exec_p.bind(
            *operands,
            out_avals=tuple(out_avals),
            in_names=tuple(all_in_names),
            out_names=tuple(out_names),
            lowering_input_output_aliases=(),
            sim_require_finite=True,
            sim_require_nnan=True,
            nc=nc,
        )
        return tuple(outs)

    devices = jax.devices()[:N_CORES]
    mesh = Mesh(np.asarray(devices), ("core",))
    nin = n_params + len(out_names)
    fn = jax.jit(shard_map(_body, mesh=mesh,
                           in_specs=(PartitionSpec("core"),) * nin,
                           out_specs=(PartitionSpec("core"),) * len(out_names),
                           check_rep=False))
    sh = NamedSharding(mesh, PartitionSpec("core"))
    dev_args = [
        jax.device_put(
            np.concatenate([np.asarray(in_maps[c][nm]) for c in range(N_CORES)],
                           axis=0), sh)
        for nm in in_names
    ] + [
        jax.device_put(np.zeros((N_CORES * z.shape[0], *z.shape[1:]), z.dtype), sh)
        for z in zero_outs
    ]

    def run_once():
        outs = fn(*dev_args)
        jax.block_until_ready(outs)
        return [
            {nm: np.asarray(outs[i]).reshape(N_CORES, *out_avals[i].shape)[c]
             for i, nm in enumerate(out_names)}
            for c in range(N_CORES)
        ]

    def time_iters(n):
        ts = []
        for _ in range(n):
            t0 = time.perf_counter()
            jax.block_until_ready(fn(*dev_args))
            ts.append(time.perf_counter() - t0)
        return ts

    return run_once, time_iters


def kernel(q, k, v, mask, Wq, bq, Wk, bk, Wv, bv, Wo, bo):
    from concourse.bass_utils import run_bass_kernel_spmd

    q, k, v = (np.asarray(x) for x in (q, k, v))
    mask = np.asarray(mask)
    in_maps = _host_prep(q, k, v, mask, np.asarray(Wq), np.asarray(bq),
                         np.asarray(Wk), np.asarray(bk), np.asarray(Wv),
                         np.asarray(bv), np.asarray(Wo))
    key = (("bq" in in_maps[0]), ("bk" in in_maps[0]), ("bv" in in_maps[0]))
    if key not in _cached:
        _cached[key] = _build_program(*key)
    nc = _cached[key]

    trace = bool(int(os.environ.get("KERNEL_TRACE", "0")))
    res = run_bass_kernel_spmd(nc, in_maps, list(range(N_CORES)), trace=trace)
    kernel.last_results = res

    y = np.zeros((T, D), np.float64)
    for i in range(N_CORES):
        y += res.results[i]["y"].astype(np.float64)
    y = (y + np.asarray(bo).astype(np.float64)).astype(np.float32)
    return y.reshape(B, L, D)
